# revision 2
# baseline (speedup 1.0000x reference)
"""DeepLSTM Trainium2 Bass kernel, v2: transposed (gate-dim on partitions).

Layout: all recurrent state kept transposed — h^T/c^T as [128 partitions
(hidden%128), (ktile, batch)] — so every elementwise/activation op runs on
128 partitions instead of 4. Gate pre-activations z^T are computed
weight-stationary: out[128 gate-dims, 4 batch] = W_tile.T @ h^T_tile with
the weight as the (stationary) lhsT, so no per-step transposes exist at all.

Per core: 4 batch rows; layer 2 of the reference is dead code (output only
needs c0|h0|c1) so only layers 0 and 1 are computed.

Gate order is host-permuted from reference (i,j,f,o) to (i,f,o,j) and the
j columns are pre-scaled by 2 so tanh(zj) = 2*sigmoid(2*zj)-1 needs only
one sigmoid pass over all 32 psum columns plus a cheap DVE 2x-1.
"""
import sys
from contextlib import ExitStack

sys.path.insert(0, "/opt/trn_rl_repo")

import concourse.bacc as bacc
import concourse.bass as bass
import concourse.mybir as mybir
import concourse.tile as tile
from concourse.masks import make_identity

F32 = mybir.dt.float32
F32R = mybir.dt.float32r
BF16 = mybir.dt.bfloat16
I32 = mybir.dt.int32
MULT = mybir.AluOpType.mult
ADD = mybir.AluOpType.add
SIG = mybir.ActivationFunctionType.Sigmoid
TANH = mybir.ActivationFunctionType.Tanh

H, G, OUT = 256, 1024, 768
Bs = 4            # batch rows per core
NKT = 2           # H / 128
MB = 8            # G / 128
VC = 8192         # compact per-core embedding table rows (>= T_pad*Bs)


def build(T_pad, S, n_chunks, weights, wh_dt=BF16, emb_dt=BF16):
    """S steps per chunk (S*4 tokens, multiple of 32); n_chunks even.
    weights: dict of host-prepped arrays (w0x/w1x f32, w0h/w1m/w1h bf16),
    embedded into the NEFF as constants (loaded to HBM at model load)."""
    assert S % 32 == 0 and n_chunks * S == T_pad and n_chunks % 2 == 0
    assert n_chunks >= 4

    nc = bacc.Bacc("TRN2", target_bir_lowering=False, debug=False)
    tok = nc.dram_tensor("tok", [T_pad * Bs], I32, kind="ExternalInput")
    nst = nc.dram_tensor("nst", [Bs], I32, kind="ExternalInput")
    emb = nc.dram_tensor("emb", [VC, H], emb_dt, kind="ExternalInput")
    # x-projection weights f32 (bitcast f32r), block (kt*8+mb) at cols *128;
    # recurrent weights wh_dt, block (mb*2+kt) at cols *128
    w_dr = {k: nc.inline_tensor(weights[k], name=k)
            for k in ("w0x", "w1x", "w0h", "w1m", "w1h")}
    out = nc.dram_tensor("out", [Bs, OUT], F32, kind="ExternalOutput")
    st_rm = nc.dram_tensor("st_rm", [T_pad * Bs, OUT], F32)

    with tile.TileContext(nc) as tc, ExitStack() as ctx:
        const_p = ctx.enter_context(tc.tile_pool(name="const", bufs=1))
        wp = ctx.enter_context(tc.tile_pool(name="wp", bufs=1))
        slab_p = ctx.enter_context(tc.tile_pool(name="slab", bufs=1))
        idx_p = ctx.enter_context(tc.tile_pool(name="idx", bufs=2))
        xg_p = ctx.enter_context(tc.tile_pool(name="xg", bufs=2))
        xT_p = ctx.enter_context(tc.tile_pool(name="xT", bufs=2))
        zs_p = ctx.enter_context(tc.tile_pool(name="zs", bufs=6))
        sm_p = ctx.enter_context(tc.tile_pool(name="sm", bufs=12))
        rm_p = ctx.enter_context(tc.tile_pool(name="rm", bufs=2))
        pz0_p = ctx.enter_context(tc.tile_pool(name="pz0", bufs=4, space="PSUM"))
        pz1_p = ctx.enter_context(tc.tile_pool(name="pz1", bufs=3, space="PSUM"))
        pxt_p = ctx.enter_context(tc.tile_pool(name="pxt", bufs=1, space="PSUM"))

        # ---- constants ----
        ident = const_p.tile([128, 128], F32)
        make_identity(nc, ident[:])
        ident_e = const_p.tile([128, 128], emb_dt)
        nc.vector.tensor_copy(ident_e[:], ident[:])

        w_sb = {}
        for name in ("w0x", "w1x", "w0h", "w1m", "w1h"):
            t = wp.tile([128, NKT * MB * 128], wh_dt, tag=name, name=name)
            nc.sync.dma_start(t[:], w_dr[name][:, :])
            w_sb[name] = t

        # persistent recurrent carry (state at last step of previous chunk),
        # split per layer so the next chunk's L0 never waits on L1's tail
        carry_c0 = const_p.tile([128, NKT, Bs], F32)
        carry_c1 = const_p.tile([128, NKT, Bs], F32)
        carry_h0 = const_p.tile([128, NKT, Bs], wh_dt)
        carry_h1 = const_p.tile([128, NKT, Bs], wh_dt)
        for t in (carry_c0, carry_c1, carry_h0, carry_h1):
            nc.vector.memset(t[:], 0.0)

        # chunk-persistent double buffers.
        # slab layouts: [128, block, S*Bs] with the (step, batch) index
        # innermost so matmul/transpose operands are single-free-dim slices.
        def mk_slabs(sfx):
            xT = slab_p.tile([128, NKT, S * Bs], wh_dt, tag="xT" + sfx,
                             name="xT" + sfx)  # x^T for the chunk
            cs = slab_p.tile([128, 4, S * Bs], F32, tag="cs" + sfx,
                             name="cs" + sfx)   # c0kt0,c0kt1,c1kt0,c1kt1
            hs = slab_p.tile([128, 4, S * Bs], wh_dt, tag="hs" + sfx,
                             name="hs" + sfx)   # h0kt0,h0kt1,h1kt0,h1kt1
            return xT, cs, hs
        bufA = mk_slabs("A")
        bufB = mk_slabs("B")

        def bulk(koff, xT):
            """Gather + transpose the chunk's embeddings into the xT slab.
            The x-projection itself happens inside each step's matmul
            group (x terms lead each group and pre-fill PSUM early)."""
            for tb in range(S * Bs // 128):
                idx = idx_p.tile([128, 1], I32, tag="idx", name="idx")
                nc.sync.dma_start(idx[:, 0:1],
                                  tok[bass.ds(koff * Bs + tb * 128, 128)])
                xg = xg_p.tile([128, H], emb_dt, tag="xg", name="xg")
                nc.gpsimd.indirect_dma_start(
                    out=xg[:], out_offset=None, in_=emb[:],
                    in_offset=bass.IndirectOffsetOnAxis(ap=idx[:, 0:1], axis=0))
                for kt in range(NKT):
                    pxt = pxt_p.tile([128, 128], emb_dt, tag="pxt",
                                     name="pxtb")
                    nc.tensor.transpose(pxt[:], xg[:, kt * 128:(kt + 1) * 128],
                                        ident_e[:])
                    nc.vector.tensor_copy(
                        xT[:, kt, tb * 128:(tb + 1) * 128], pxt[:])

        def xphase(pz_tag, wxname, xT, s):
            """Open step s's PSUM group with the x-projection matmuls.
            Emitted ahead of the recurrent phase so the (FIFO) PE runs them
            while the h matmuls of earlier steps are still blocked."""
            sl = slice(s * Bs, (s + 1) * Bs)
            pool = pz0_p if pz_tag == "pz0" else pz1_p
            pz = pool.tile([128, MB, Bs], F32, tag=pz_tag, name=pz_tag)
            # ONE start for the whole tile: start=True marks the entire 2KB
            # psum bank pending-zero, so each slice's first write overwrites
            # (initializes) and every later write accumulates.
            for mb in range(MB):
                for kt in range(NKT):
                    nc.tensor.matmul(
                        pz[:, mb, :],
                        lhsT=w_sb[wxname][:, (mb * NKT + kt) * 128:
                                          (mb * NKT + kt + 1) * 128],
                        rhs=xT[:, kt, sl],
                        start=(mb == 0 and kt == 0), stop=False,
                        skip_group_check=(mb or kt) != 0)
            return pz

        def cell(s, pz, zs_tag, terms, cprev, cout, hout_b):
            """Recurrent phase of one LSTM cell (transposed layout).
            terms: list of (w_name, rhs_fn) with rhs_fn(kt) -> [128, Bs] AP;
            accumulates onto the x-projection already in pz."""
            nmm = len(terms) * NKT
            for mb in range(MB):
                mi = 0
                for wname, rhs_fn in terms:
                    for kt in range(NKT):
                        mi += 1
                        last = (mb == MB - 1 and mi == nmm)
                        nc.tensor.matmul(
                            pz[:, mb, :],
                            lhsT=w_sb[wname][:, (mb * NKT + kt) * 128:
                                             (mb * NKT + kt + 1) * 128],
                            rhs=rhs_fn(kt),
                            start=False, stop=last,
                            skip_group_check=not last)
            zs = zs_p.tile([128, MB, Bs], F32, tag=zs_tag, name=zs_tag)
            nc.scalar.activation(zs[:, :, :], pz[:, :, :], SIG)
            si, sf, so = zs[:, 0:2, :], zs[:, 2:4, :], zs[:, 4:6, :]
            tj = sm_p.tile([128, NKT, Bs], F32, tag="tj", name="tj")
            nc.vector.tensor_scalar(out=tj[:, :, :], in0=zs[:, 6:8, :],
                                    scalar1=2.0, scalar2=-1.0,
                                    op0=MULT, op1=ADD)
            v = sm_p.tile([128, NKT, Bs], F32, tag="v", name="v")
            nc.vector.tensor_tensor(v[:, :, :], cprev, sf, op=MULT)
            u = sm_p.tile([128, NKT, Bs], F32, tag="u", name="u")
            nc.vector.tensor_tensor(u[:, :, :], si, tj[:, :, :], op=MULT)
            nc.vector.tensor_tensor(cout, u[:, :, :], v[:, :, :], op=ADD)
            tc_ = sm_p.tile([128, NKT, Bs], F32, tag="tc", name="tc")
            nc.scalar.activation(tc_[:, :, :], cout, TANH)
            nc.vector.tensor_tensor(hout_b, tc_[:, :, :], so, op=MULT)

        LAG = 2

        def cell1(s, pz, cs, hs):
            """Layer-1 recurrent phase for step s (emitted LAG steps late so
            its ops are always dep-ready when the FIFO engines reach them)."""
            sl = slice(s * Bs, (s + 1) * Bs)
            if s == 0:
                c1p = carry_c1[:, :, :]
                h1p = lambda kt: carry_h1[:, kt, :]
            else:
                pv = slice((s - 1) * Bs, s * Bs)
                c1p = cs[:, 2:4, pv]
                h1p = lambda kt, pv=pv: hs[:, 2 + kt, pv]
            h0c = lambda kt, sl=sl: hs[:, kt, sl]
            cell(s, pz, "zs1",
                 [("w1m", h0c), ("w1h", h1p)],
                 c1p, cs[:, 2:4, sl], hs[:, 2:4, sl])

        P0 = 3  # x-phase lookahead for layer 0

        def steps_chunk(koff, xT, cs, hs):
            q0, q1 = [], []
            for s in range(P0):
                q0.append(xphase("pz0", "w0x", xT, s))
            for s in range(S):
                sl = slice(s * Bs, (s + 1) * Bs)
                if s == 0:
                    c0p = carry_c0[:, :, :]
                    h0p = lambda kt: carry_h0[:, kt, :]
                else:
                    pv = slice((s - 1) * Bs, s * Bs)
                    c0p = cs[:, 0:2, pv]
                    h0p = lambda kt, pv=pv: hs[:, kt, pv]
                if s + P0 < S:
                    q0.append(xphase("pz0", "w0x", xT, s + P0))
                cell(s, q0.pop(0), "zs0",
                     [("w0h", h0p)],
                     c0p, cs[:, 0:2, sl], hs[:, 0:2, sl])
                q1.append(xphase("pz1", "w1x", xT, s))
                if s >= LAG:
                    cell1(s - LAG, q1.pop(0), cs, hs)
            for s in range(S - LAG, S):
                cell1(s, q1.pop(0), cs, hs)
            # carry into persistent tiles (split per layer)
            lastc = slice((S - 1) * Bs, S * Bs)
            nc.vector.tensor_copy(carry_c0[:], cs[:, 0:2, lastc])
            nc.vector.tensor_copy(carry_h0[:], hs[:, 0:2, lastc])
            nc.vector.tensor_copy(carry_c1[:], cs[:, 2:4, lastc])
            nc.vector.tensor_copy(carry_h1[:], hs[:, 2:4, lastc])
            # extraction: transpose c0/h0/c1 to row-major [4*S, OUT] rows
            for tb in range(S * Bs // 128):
                tsl = slice(tb * 128, (tb + 1) * 128)
                rm = rm_p.tile([128, OUT], F32, tag="rm", name="rm")
                srcs = []
                for kt in range(NKT):
                    srcs.append((cs[:, kt, tsl], kt * 128, F32))
                    srcs.append((hs[:, kt, tsl], H + kt * 128, wh_dt))
                    srcs.append((cs[:, 2 + kt, tsl], 2 * H + kt * 128, F32))
                for src, coff, sdt in srcs:
                    pxt = pxt_p.tile([128, 128], sdt, tag="pxt", name="pxt")
                    idn = ident[:] if sdt == F32 else ident_e[:]
                    nc.tensor.transpose(pxt[:], src, idn)
                    nc.vector.tensor_copy(rm[:, coff:coff + 128], pxt[:])
                nc.sync.dma_start(
                    st_rm[bass.ds((koff + tb * 32) * Bs, 128), :], rm[:])

        # ---- software-pipelined chunk loop ----
        bulk(0, bufA[0])
        with tc.For_i(0, (n_chunks - 2) * S, 2 * S,
                      hint_engines=tuple(mybir.ALL_ENGINES)) as k:
            bulk(k + S, bufB[0])
            steps_chunk(k, *bufA)
            bulk(k + 2 * S, bufA[0])
            steps_chunk(k + S, *bufB)
        bulk((n_chunks - 1) * S, bufB[0])
        steps_chunk((n_chunks - 2) * S, *bufA)
        steps_chunk((n_chunks - 1) * S, *bufB)

        # ---- output extraction ----
        nst_sb = const_p.tile([Bs, 1], I32)
        nc.sync.dma_start(nst_sb[:, 0:1], nst[:])
        iota_t = const_p.tile([Bs, 1], I32)
        nc.gpsimd.iota(iota_t[:], pattern=[[0, 1]], base=0,
                       channel_multiplier=1)
        ridx = const_p.tile([Bs, 1], I32)
        nc.vector.tensor_scalar(out=ridx[:], in0=nst_sb[:], scalar1=Bs,
                                scalar2=None, op0=MULT)
        nc.vector.tensor_tensor(ridx[:], ridx[:], iota_t[:], op=ADD)
        out_sb = const_p.tile([Bs, OUT], F32)
        nc.gpsimd.indirect_dma_start(
            out=out_sb[:], out_offset=None, in_=st_rm[:],
            in_offset=bass.IndirectOffsetOnAxis(ap=ridx[:, 0:1], axis=0))
        nc.sync.dma_start(out[:], out_sb[:])

    nc.compile()
    return nc


# ---------------------------------------------------------------------------
# Host-side entry point
# ---------------------------------------------------------------------------
import numpy as np

N_CORES = 8
_PROGRAM_CACHE = {}
_last_in_maps = None
_last_nc = None


def _plan(T):
    S = 128
    n_chunks = max(4, -(-T // S))
    if n_chunks % 2:
        n_chunks += 1
    return S * n_chunks, S, n_chunks


def _get_program(T_pad, S, n_chunks, weights, wkey):
    key = (T_pad, S, n_chunks, wkey)
    if key not in _PROGRAM_CACHE:
        _PROGRAM_CACHE[key] = build(T_pad, S, n_chunks, weights)
    return _PROGRAM_CACHE[key]


def _block_layout(w, kt_major):
    """[256, 1024] -> [128, 16*128] with 128x128 blocks at col offsets.
    kt_major: block index = kt*8+mb, else mb*2+kt."""
    t = w.reshape(NKT, 128, MB, 128)          # kt, p, mb, j
    if kt_major:
        t = t.transpose(1, 0, 2, 3)           # p, kt, mb, j
    else:
        t = t.transpose(1, 2, 0, 3)           # p, mb, kt, j
    return np.ascontiguousarray(t.reshape(128, NKT * MB * 128))


def kernel(inputs, nstarts, emb, W0, b0, W1, b1, W2, b2, _run_kwargs=None,
           _return_raw=False):
    inputs = np.asarray(inputs)
    nstarts = np.asarray(nstarts)
    emb = np.asarray(emb, np.float32)
    B, T = inputs.shape
    assert B == N_CORES * Bs
    T_pad, S, n_chunks = _plan(T)

    W0 = np.asarray(W0, np.float32)
    W1 = np.asarray(W1, np.float32)
    b0 = np.asarray(b0, np.float32)
    b1 = np.asarray(b1, np.float32)
    assert not b0.any() and not b1.any(), "nonzero biases unsupported"
    # gate perm (i,j,f,o) -> (i,f,o,j); j cols scaled by 2 (tanh via sigmoid)
    perm = np.r_[0:H, 2 * H:3 * H, 3 * H:4 * H, H:2 * H]
    jscale = np.ones(G, np.float32)
    jscale[3 * H:] = 2.0

    def prep(wrows, kt_major):
        return _block_layout(np.ascontiguousarray(wrows[:, perm] * jscale),
                             kt_major)

    import ml_dtypes
    w_maps = {
        "w0x": prep(W0[0:H], False).astype(ml_dtypes.bfloat16),
        "w0h": prep(W0[H:2 * H], False).astype(ml_dtypes.bfloat16),
        "w1x": prep(W1[0:H], False).astype(ml_dtypes.bfloat16),
        "w1m": prep(W1[H:2 * H], False).astype(ml_dtypes.bfloat16),
        "w1h": prep(W1[2 * H:3 * H], False).astype(ml_dtypes.bfloat16),
    }
    import hashlib
    wkey = hashlib.sha1(W0.tobytes() + W1.tobytes()).hexdigest()[:16]

    tt_by_batch = np.zeros(B, np.int64)
    tt_by_batch[nstarts[:, 1].astype(np.int64)] = nstarts[:, 0].astype(np.int64)

    in_maps = []
    for c in range(N_CORES):
        shard = inputs[c * Bs:(c + 1) * Bs].astype(np.int64)   # [Bs, T]
        tokp = np.zeros((T_pad, Bs), np.int64)
        tokp[:T, :] = shard.T
        uniq, inv = np.unique(tokp, return_inverse=True)
        assert len(uniq) <= VC
        emb_c = np.zeros((VC, H), ml_dtypes.bfloat16)
        emb_c[:len(uniq)] = emb[uniq].astype(ml_dtypes.bfloat16)
        in_maps.append(dict(
            tok=np.ascontiguousarray(inv.reshape(T_pad * Bs).astype(np.int32)),
            nst=np.ascontiguousarray(
                tt_by_batch[c * Bs:(c + 1) * Bs].astype(np.int32)),
            emb=emb_c,
        ))

    global _last_in_maps, _last_nc
    _last_in_maps = in_maps
    nc = _get_program(T_pad, S, n_chunks, w_maps, wkey)
    _last_nc = nc
    from concourse.bass_utils import run_bass_kernel_spmd
    kw = dict(_run_kwargs or {})
    res = run_bass_kernel_spmd(nc, in_maps, list(range(N_CORES)), **kw)
    per_batch = np.concatenate([res.results[c]["out"] for c in range(N_CORES)],
                               axis=0)                          # [B, OUT]
    full = per_batch[nstarts[:, 1].astype(np.int64)].astype(np.float32)
    if _return_raw:
        return full, res
    return full


# revision 3
# speedup vs baseline: 1.0021x; 1.0021x over previous
"""DeepLSTM Trainium2 Bass kernel, v2: transposed (gate-dim on partitions).

Layout: all recurrent state kept transposed — h^T/c^T as [128 partitions
(hidden%128), (ktile, batch)] — so every elementwise/activation op runs on
128 partitions instead of 4. Gate pre-activations z^T are computed
weight-stationary: out[128 gate-dims, 4 batch] = W_tile.T @ h^T_tile with
the weight as the (stationary) lhsT, so no per-step transposes exist at all.

Per core: 4 batch rows; layer 2 of the reference is dead code (output only
needs c0|h0|c1) so only layers 0 and 1 are computed.

Gate order is host-permuted from reference (i,j,f,o) to (i,f,o,j) and the
j columns are pre-scaled by 2 so tanh(zj) = 2*sigmoid(2*zj)-1 needs only
one sigmoid pass over all 32 psum columns plus a cheap DVE 2x-1.
"""
import sys
from contextlib import ExitStack

sys.path.insert(0, "/opt/trn_rl_repo")

import concourse.bacc as bacc
import concourse.bass as bass
import concourse.mybir as mybir
import concourse.tile as tile
from concourse.masks import make_identity

F32 = mybir.dt.float32
F32R = mybir.dt.float32r
BF16 = mybir.dt.bfloat16
I32 = mybir.dt.int32
MULT = mybir.AluOpType.mult
ADD = mybir.AluOpType.add
SIG = mybir.ActivationFunctionType.Sigmoid
TANH = mybir.ActivationFunctionType.Tanh

H, G, OUT = 256, 1024, 768
Bs = 4            # batch rows per core
NKT = 2           # H / 128
MB = 8            # G / 128
VC = 8192         # compact per-core embedding table rows (>= T_pad*Bs)


def build(T_pad, S, n_chunks, weights, wh_dt=BF16, emb_dt=BF16):
    """S steps per chunk (S*4 tokens, multiple of 32); n_chunks even.
    weights: dict of host-prepped arrays (w0x/w1x f32, w0h/w1m/w1h bf16),
    embedded into the NEFF as constants (loaded to HBM at model load)."""
    assert S % 32 == 0 and n_chunks * S == T_pad and n_chunks % 2 == 0
    assert n_chunks >= 4

    nc = bacc.Bacc("TRN2", target_bir_lowering=False, debug=False)
    tok = nc.dram_tensor("tok", [T_pad * Bs], I32, kind="ExternalInput")
    nst = nc.dram_tensor("nst", [Bs], I32, kind="ExternalInput")
    emb = nc.dram_tensor("emb", [VC, H], emb_dt, kind="ExternalInput")
    # x-projection weights f32 (bitcast f32r), block (kt*8+mb) at cols *128;
    # recurrent weights wh_dt, block (mb*2+kt) at cols *128
    w_dr = {k: nc.inline_tensor(weights[k], name=k)
            for k in ("w0x", "w1x", "w0h", "w1m", "w1h")}
    out = nc.dram_tensor("out", [Bs, OUT], F32, kind="ExternalOutput")
    st_rm = nc.dram_tensor("st_rm", [T_pad * Bs, OUT], F32)

    with tile.TileContext(nc) as tc, ExitStack() as ctx:
        const_p = ctx.enter_context(tc.tile_pool(name="const", bufs=1))
        wp = ctx.enter_context(tc.tile_pool(name="wp", bufs=1))
        slab_p = ctx.enter_context(tc.tile_pool(name="slab", bufs=1))
        idx_p = ctx.enter_context(tc.tile_pool(name="idx", bufs=2))
        xg_p = ctx.enter_context(tc.tile_pool(name="xg", bufs=2))
        xT_p = ctx.enter_context(tc.tile_pool(name="xT", bufs=2))
        zs_p = ctx.enter_context(tc.tile_pool(name="zs", bufs=6))
        sm_p = ctx.enter_context(tc.tile_pool(name="sm", bufs=12))
        rm_p = ctx.enter_context(tc.tile_pool(name="rm", bufs=2))
        pz0_p = ctx.enter_context(tc.tile_pool(name="pz0", bufs=4, space="PSUM"))
        pz1_p = ctx.enter_context(tc.tile_pool(name="pz1", bufs=3, space="PSUM"))
        pxt_p = ctx.enter_context(tc.tile_pool(name="pxt", bufs=1, space="PSUM"))

        # ---- constants ----
        ident = const_p.tile([128, 128], F32)
        make_identity(nc, ident[:])
        ident_e = const_p.tile([128, 128], emb_dt)
        nc.vector.tensor_copy(ident_e[:], ident[:])

        w_sb = {}
        for name in ("w0x", "w1x", "w0h", "w1m", "w1h"):
            t = wp.tile([128, NKT * MB * 128], wh_dt, tag=name, name=name)
            nc.sync.dma_start(t[:], w_dr[name][:, :])
            w_sb[name] = t

        # persistent recurrent carry (state at last step of previous chunk),
        # split per layer so the next chunk's L0 never waits on L1's tail
        carry_c0 = const_p.tile([128, NKT, Bs], F32)
        carry_c1 = const_p.tile([128, NKT, Bs], F32)
        carry_h0 = const_p.tile([128, NKT, Bs], wh_dt)
        carry_h1 = const_p.tile([128, NKT, Bs], wh_dt)
        for t in (carry_c0, carry_c1, carry_h0, carry_h1):
            nc.vector.memset(t[:], 0.0)

        # chunk-persistent double buffers.
        # slab layouts: [128, block, S*Bs] with the (step, batch) index
        # innermost so matmul/transpose operands are single-free-dim slices.
        def mk_slabs(sfx):
            xT = slab_p.tile([128, NKT, S * Bs], wh_dt, tag="xT" + sfx,
                             name="xT" + sfx)  # x^T for the chunk
            cs = slab_p.tile([128, 4, S * Bs], F32, tag="cs" + sfx,
                             name="cs" + sfx)   # c0kt0,c0kt1,c1kt0,c1kt1
            hs = slab_p.tile([128, 4, S * Bs], wh_dt, tag="hs" + sfx,
                             name="hs" + sfx)   # h0kt0,h0kt1,h1kt0,h1kt1
            return xT, cs, hs
        bufA = mk_slabs("A")
        bufB = mk_slabs("B")

        def bulk(koff, xT):
            """Gather + transpose the chunk's embeddings into the xT slab.
            The x-projection itself happens inside each step's matmul
            group (x terms lead each group and pre-fill PSUM early)."""
            for tb in range(S * Bs // 128):
                idx = idx_p.tile([128, 1], I32, tag="idx", name="idx")
                nc.sync.dma_start(idx[:, 0:1],
                                  tok[bass.ds(koff * Bs + tb * 128, 128)])
                xg = xg_p.tile([128, H], emb_dt, tag="xg", name="xg")
                nc.gpsimd.indirect_dma_start(
                    out=xg[:], out_offset=None, in_=emb[:],
                    in_offset=bass.IndirectOffsetOnAxis(ap=idx[:, 0:1], axis=0))
                for kt in range(NKT):
                    pxt = pxt_p.tile([128, 128], emb_dt, tag="pxt",
                                     name="pxtb")
                    nc.tensor.transpose(pxt[:], xg[:, kt * 128:(kt + 1) * 128],
                                        ident_e[:])
                    nc.vector.tensor_copy(
                        xT[:, kt, tb * 128:(tb + 1) * 128], pxt[:])

        def xphase(pz_tag, wxname, xT, s):
            """Open step s's PSUM group with the x-projection matmuls.
            Emitted ahead of the recurrent phase so the (FIFO) PE runs them
            while the h matmuls of earlier steps are still blocked."""
            sl = slice(s * Bs, (s + 1) * Bs)
            pool = pz0_p if pz_tag == "pz0" else pz1_p
            pz = pool.tile([128, MB, Bs], F32, tag=pz_tag, name=pz_tag)
            # ONE start for the whole tile: start=True marks the entire 2KB
            # psum bank pending-zero, so each slice's first write overwrites
            # (initializes) and every later write accumulates.
            for mb in range(MB):
                for kt in range(NKT):
                    nc.tensor.matmul(
                        pz[:, mb, :],
                        lhsT=w_sb[wxname][:, (mb * NKT + kt) * 128:
                                          (mb * NKT + kt + 1) * 128],
                        rhs=xT[:, kt, sl],
                        start=(mb == 0 and kt == 0), stop=False,
                        skip_group_check=(mb or kt) != 0)
            return pz

        def cell(s, pz, zs_tag, terms, cprev, cout, hout_b, ceng=None):
            """Recurrent phase of one LSTM cell (transposed layout).
            terms: list of (w_name, rhs_fn) with rhs_fn(kt) -> [128, Bs] AP;
            accumulates onto the x-projection already in pz. ceng: engine
            for the c-update trio (L1 uses the idle GpSimd so its ops never
            queue ahead of L0's in the DVE FIFO)."""
            ceng = ceng or nc.vector
            nmm = len(terms) * NKT
            for mb in range(MB):
                mi = 0
                for wname, rhs_fn in terms:
                    for kt in range(NKT):
                        mi += 1
                        last = (mb == MB - 1 and mi == nmm)
                        nc.tensor.matmul(
                            pz[:, mb, :],
                            lhsT=w_sb[wname][:, (mb * NKT + kt) * 128:
                                             (mb * NKT + kt + 1) * 128],
                            rhs=rhs_fn(kt),
                            start=False, stop=last,
                            skip_group_check=not last)
            zs = zs_p.tile([128, MB, Bs], F32, tag=zs_tag, name=zs_tag)
            nc.scalar.activation(zs[:, :, :], pz[:, :, :], SIG)
            si, sf, so = zs[:, 0:2, :], zs[:, 2:4, :], zs[:, 4:6, :]
            tj = sm_p.tile([128, NKT, Bs], F32, tag="tj", name="tj")
            nc.vector.tensor_scalar(out=tj[:, :, :], in0=zs[:, 6:8, :],
                                    scalar1=2.0, scalar2=-1.0,
                                    op0=MULT, op1=ADD)
            v = sm_p.tile([128, NKT, Bs], F32, tag="v", name="v")
            ceng.tensor_tensor(v[:, :, :], cprev, sf, op=MULT)
            u = sm_p.tile([128, NKT, Bs], F32, tag="u", name="u")
            ceng.tensor_tensor(u[:, :, :], si, tj[:, :, :], op=MULT)
            ceng.tensor_tensor(cout, u[:, :, :], v[:, :, :], op=ADD)
            tc_ = sm_p.tile([128, NKT, Bs], F32, tag="tc", name="tc")
            nc.scalar.activation(tc_[:, :, :], cout, TANH)
            nc.vector.tensor_tensor(hout_b, tc_[:, :, :], so, op=MULT)

        LAG = 2

        def cell1(s, pz, cs, hs):
            """Layer-1 recurrent phase for step s (emitted LAG steps late so
            its ops are always dep-ready when the FIFO engines reach them)."""
            sl = slice(s * Bs, (s + 1) * Bs)
            if s == 0:
                c1p = carry_c1[:, :, :]
                h1p = lambda kt: carry_h1[:, kt, :]
            else:
                pv = slice((s - 1) * Bs, s * Bs)
                c1p = cs[:, 2:4, pv]
                h1p = lambda kt, pv=pv: hs[:, 2 + kt, pv]
            h0c = lambda kt, sl=sl: hs[:, kt, sl]
            cell(s, pz, "zs1",
                 [("w1m", h0c), ("w1h", h1p)],
                 c1p, cs[:, 2:4, sl], hs[:, 2:4, sl], ceng=nc.gpsimd)

        P0 = 3  # x-phase lookahead for layer 0

        def steps_chunk(koff, xT, cs, hs):
            q0, q1 = [], []
            for s in range(P0):
                q0.append(xphase("pz0", "w0x", xT, s))
            for s in range(S):
                sl = slice(s * Bs, (s + 1) * Bs)
                if s == 0:
                    c0p = carry_c0[:, :, :]
                    h0p = lambda kt: carry_h0[:, kt, :]
                else:
                    pv = slice((s - 1) * Bs, s * Bs)
                    c0p = cs[:, 0:2, pv]
                    h0p = lambda kt, pv=pv: hs[:, kt, pv]
                if s + P0 < S:
                    q0.append(xphase("pz0", "w0x", xT, s + P0))
                cell(s, q0.pop(0), "zs0",
                     [("w0h", h0p)],
                     c0p, cs[:, 0:2, sl], hs[:, 0:2, sl])
                q1.append(xphase("pz1", "w1x", xT, s))
                if s >= LAG:
                    cell1(s - LAG, q1.pop(0), cs, hs)
            for s in range(S - LAG, S):
                cell1(s, q1.pop(0), cs, hs)
            # carry into persistent tiles (split per layer)
            lastc = slice((S - 1) * Bs, S * Bs)
            nc.vector.tensor_copy(carry_c0[:], cs[:, 0:2, lastc])
            nc.vector.tensor_copy(carry_h0[:], hs[:, 0:2, lastc])
            nc.vector.tensor_copy(carry_c1[:], cs[:, 2:4, lastc])
            nc.vector.tensor_copy(carry_h1[:], hs[:, 2:4, lastc])
            # extraction: transpose c0/h0/c1 to row-major [4*S, OUT] rows
            for tb in range(S * Bs // 128):
                tsl = slice(tb * 128, (tb + 1) * 128)
                rm = rm_p.tile([128, OUT], F32, tag="rm", name="rm")
                srcs = []
                for kt in range(NKT):
                    srcs.append((cs[:, kt, tsl], kt * 128, F32))
                    srcs.append((hs[:, kt, tsl], H + kt * 128, wh_dt))
                    srcs.append((cs[:, 2 + kt, tsl], 2 * H + kt * 128, F32))
                for src, coff, sdt in srcs:
                    pxt = pxt_p.tile([128, 128], sdt, tag="pxt", name="pxt")
                    idn = ident[:] if sdt == F32 else ident_e[:]
                    nc.tensor.transpose(pxt[:], src, idn)
                    nc.vector.tensor_copy(rm[:, coff:coff + 128], pxt[:])
                nc.sync.dma_start(
                    st_rm[bass.ds((koff + tb * 32) * Bs, 128), :], rm[:])

        # ---- software-pipelined chunk loop ----
        bulk(0, bufA[0])
        with tc.For_i(0, (n_chunks - 2) * S, 2 * S,
                      hint_engines=tuple(mybir.ALL_ENGINES)) as k:
            bulk(k + S, bufB[0])
            steps_chunk(k, *bufA)
            bulk(k + 2 * S, bufA[0])
            steps_chunk(k + S, *bufB)
        bulk((n_chunks - 1) * S, bufB[0])
        steps_chunk((n_chunks - 2) * S, *bufA)
        steps_chunk((n_chunks - 1) * S, *bufB)

        # ---- output extraction ----
        nst_sb = const_p.tile([Bs, 1], I32)
        nc.sync.dma_start(nst_sb[:, 0:1], nst[:])
        iota_t = const_p.tile([Bs, 1], I32)
        nc.gpsimd.iota(iota_t[:], pattern=[[0, 1]], base=0,
                       channel_multiplier=1)
        ridx = const_p.tile([Bs, 1], I32)
        nc.vector.tensor_scalar(out=ridx[:], in0=nst_sb[:], scalar1=Bs,
                                scalar2=None, op0=MULT)
        nc.vector.tensor_tensor(ridx[:], ridx[:], iota_t[:], op=ADD)
        out_sb = const_p.tile([Bs, OUT], F32)
        nc.gpsimd.indirect_dma_start(
            out=out_sb[:], out_offset=None, in_=st_rm[:],
            in_offset=bass.IndirectOffsetOnAxis(ap=ridx[:, 0:1], axis=0))
        nc.sync.dma_start(out[:], out_sb[:])

    nc.compile()
    return nc


# ---------------------------------------------------------------------------
# Host-side entry point
# ---------------------------------------------------------------------------
import numpy as np

N_CORES = 8
_PROGRAM_CACHE = {}
_last_in_maps = None
_last_nc = None


def _plan(T):
    S = 128
    n_chunks = max(4, -(-T // S))
    if n_chunks % 2:
        n_chunks += 1
    return S * n_chunks, S, n_chunks


def _get_program(T_pad, S, n_chunks, weights, wkey):
    key = (T_pad, S, n_chunks, wkey)
    if key not in _PROGRAM_CACHE:
        _PROGRAM_CACHE[key] = build(T_pad, S, n_chunks, weights)
    return _PROGRAM_CACHE[key]


def _block_layout(w, kt_major):
    """[256, 1024] -> [128, 16*128] with 128x128 blocks at col offsets.
    kt_major: block index = kt*8+mb, else mb*2+kt."""
    t = w.reshape(NKT, 128, MB, 128)          # kt, p, mb, j
    if kt_major:
        t = t.transpose(1, 0, 2, 3)           # p, kt, mb, j
    else:
        t = t.transpose(1, 2, 0, 3)           # p, mb, kt, j
    return np.ascontiguousarray(t.reshape(128, NKT * MB * 128))


def kernel(inputs, nstarts, emb, W0, b0, W1, b1, W2, b2, _run_kwargs=None,
           _return_raw=False):
    inputs = np.asarray(inputs)
    nstarts = np.asarray(nstarts)
    emb = np.asarray(emb, np.float32)
    B, T = inputs.shape
    assert B == N_CORES * Bs
    T_pad, S, n_chunks = _plan(T)

    W0 = np.asarray(W0, np.float32)
    W1 = np.asarray(W1, np.float32)
    b0 = np.asarray(b0, np.float32)
    b1 = np.asarray(b1, np.float32)
    assert not b0.any() and not b1.any(), "nonzero biases unsupported"
    # gate perm (i,j,f,o) -> (i,f,o,j); j cols scaled by 2 (tanh via sigmoid)
    perm = np.r_[0:H, 2 * H:3 * H, 3 * H:4 * H, H:2 * H]
    jscale = np.ones(G, np.float32)
    jscale[3 * H:] = 2.0

    def prep(wrows, kt_major):
        return _block_layout(np.ascontiguousarray(wrows[:, perm] * jscale),
                             kt_major)

    import ml_dtypes
    w_maps = {
        "w0x": prep(W0[0:H], False).astype(ml_dtypes.bfloat16),
        "w0h": prep(W0[H:2 * H], False).astype(ml_dtypes.bfloat16),
        "w1x": prep(W1[0:H], False).astype(ml_dtypes.bfloat16),
        "w1m": prep(W1[H:2 * H], False).astype(ml_dtypes.bfloat16),
        "w1h": prep(W1[2 * H:3 * H], False).astype(ml_dtypes.bfloat16),
    }
    import hashlib
    wkey = hashlib.sha1(W0.tobytes() + W1.tobytes()).hexdigest()[:16]

    tt_by_batch = np.zeros(B, np.int64)
    tt_by_batch[nstarts[:, 1].astype(np.int64)] = nstarts[:, 0].astype(np.int64)

    in_maps = []
    for c in range(N_CORES):
        shard = inputs[c * Bs:(c + 1) * Bs].astype(np.int64)   # [Bs, T]
        tokp = np.zeros((T_pad, Bs), np.int64)
        tokp[:T, :] = shard.T
        uniq, inv = np.unique(tokp, return_inverse=True)
        assert len(uniq) <= VC
        emb_c = np.zeros((VC, H), ml_dtypes.bfloat16)
        emb_c[:len(uniq)] = emb[uniq].astype(ml_dtypes.bfloat16)
        in_maps.append(dict(
            tok=np.ascontiguousarray(inv.reshape(T_pad * Bs).astype(np.int32)),
            nst=np.ascontiguousarray(
                tt_by_batch[c * Bs:(c + 1) * Bs].astype(np.int32)),
            emb=emb_c,
        ))

    global _last_in_maps, _last_nc
    _last_in_maps = in_maps
    nc = _get_program(T_pad, S, n_chunks, w_maps, wkey)
    _last_nc = nc
    from concourse.bass_utils import run_bass_kernel_spmd
    kw = dict(_run_kwargs or {})
    res = run_bass_kernel_spmd(nc, in_maps, list(range(N_CORES)), **kw)
    per_batch = np.concatenate([res.results[c]["out"] for c in range(N_CORES)],
                               axis=0)                          # [B, OUT]
    full = per_batch[nstarts[:, 1].astype(np.int64)].astype(np.float32)
    if _return_raw:
        return full, res
    return full


# revision 4
# speedup vs baseline: 1.0426x; 1.0404x over previous
"""DeepLSTM Trainium2 Bass kernel, v2: transposed (gate-dim on partitions).

Layout: all recurrent state kept transposed — h^T/c^T as [128 partitions
(hidden%128), (ktile, batch)] — so every elementwise/activation op runs on
128 partitions instead of 4. Gate pre-activations z^T are computed
weight-stationary: out[128 gate-dims, 4 batch] = W_tile.T @ h^T_tile with
the weight as the (stationary) lhsT, so no per-step transposes exist at all.

Per core: 4 batch rows; layer 2 of the reference is dead code (output only
needs c0|h0|c1) so only layers 0 and 1 are computed.

Gate order is host-permuted from reference (i,j,f,o) to (i,f,o,j) and the
j columns are pre-scaled by 2 so tanh(zj) = 2*sigmoid(2*zj)-1 needs only
one sigmoid pass over all 32 psum columns plus a cheap DVE 2x-1.
"""
import sys
from contextlib import ExitStack

sys.path.insert(0, "/opt/trn_rl_repo")

import concourse.bacc as bacc
import concourse.bass as bass
import concourse.mybir as mybir
import concourse.tile as tile
from concourse.masks import make_identity

F32 = mybir.dt.float32
F32R = mybir.dt.float32r
BF16 = mybir.dt.bfloat16
I32 = mybir.dt.int32
MULT = mybir.AluOpType.mult
ADD = mybir.AluOpType.add
SIG = mybir.ActivationFunctionType.Sigmoid
TANH = mybir.ActivationFunctionType.Tanh

H, G, OUT = 256, 1024, 768
Bs = 4            # batch rows per core
NKT = 2           # H / 128
MB = 8            # G / 128
VC = 8192         # compact per-core embedding table rows (>= T_pad*Bs)


def build(T_pad, S, n_chunks, weights, wh_dt=BF16, emb_dt=BF16):
    """S steps per chunk (S*4 tokens, multiple of 32); n_chunks even.
    weights: dict of host-prepped arrays (w0x/w1x f32, w0h/w1m/w1h bf16),
    embedded into the NEFF as constants (loaded to HBM at model load)."""
    assert S % 32 == 0 and n_chunks * S == T_pad and n_chunks % 2 == 0
    assert n_chunks >= 4

    nc = bacc.Bacc("TRN2", target_bir_lowering=False, debug=False)
    tok = nc.dram_tensor("tok", [T_pad * Bs], I32, kind="ExternalInput")
    nst = nc.dram_tensor("nst", [Bs], I32, kind="ExternalInput")
    emb = nc.dram_tensor("emb", [VC, H], emb_dt, kind="ExternalInput")
    # x-projection weights f32 (bitcast f32r), block (kt*8+mb) at cols *128;
    # recurrent weights wh_dt, block (mb*2+kt) at cols *128
    w_dr = {k: nc.inline_tensor(weights[k], name=k)
            for k in ("w0x", "w1x", "w0h", "w1m", "w1h")}
    out = nc.dram_tensor("out", [Bs, OUT], F32, kind="ExternalOutput")
    st_rm = nc.dram_tensor("st_rm", [T_pad * Bs, OUT], F32)

    with tile.TileContext(nc) as tc, ExitStack() as ctx:
        const_p = ctx.enter_context(tc.tile_pool(name="const", bufs=1))
        wp = ctx.enter_context(tc.tile_pool(name="wp", bufs=1))
        slab_p = ctx.enter_context(tc.tile_pool(name="slab", bufs=1))
        idx_p = ctx.enter_context(tc.tile_pool(name="idx", bufs=2))
        xg_p = ctx.enter_context(tc.tile_pool(name="xg", bufs=2))
        xT_p = ctx.enter_context(tc.tile_pool(name="xT", bufs=2))
        zs_p = ctx.enter_context(tc.tile_pool(name="zs", bufs=6))
        sm_p = ctx.enter_context(tc.tile_pool(name="sm", bufs=12))
        rm_p = ctx.enter_context(tc.tile_pool(name="rm", bufs=2))
        pz0_p = ctx.enter_context(tc.tile_pool(name="pz0", bufs=4, space="PSUM"))
        pz1_p = ctx.enter_context(tc.tile_pool(name="pz1", bufs=3, space="PSUM"))
        pxt_p = ctx.enter_context(tc.tile_pool(name="pxt", bufs=1, space="PSUM"))

        # ---- constants ----
        ident = const_p.tile([128, 128], F32)
        make_identity(nc, ident[:])
        ident_e = const_p.tile([128, 128], emb_dt)
        nc.vector.tensor_copy(ident_e[:], ident[:])

        w_sb = {}
        for name in ("w0x", "w1x", "w0h", "w1m", "w1h"):
            t = wp.tile([128, NKT * MB * 128], wh_dt, tag=name, name=name)
            nc.sync.dma_start(t[:], w_dr[name][:, :])
            w_sb[name] = t

        # persistent recurrent carry (state at last step of previous chunk),
        # split per layer so the next chunk's L0 never waits on L1's tail
        carry_c0 = const_p.tile([128, NKT, Bs], F32)
        carry_c1 = const_p.tile([128, NKT, Bs], F32)
        carry_h0 = const_p.tile([128, NKT, Bs], wh_dt)
        carry_h1 = const_p.tile([128, NKT, Bs], wh_dt)
        for t in (carry_c0, carry_c1, carry_h0, carry_h1):
            nc.vector.memset(t[:], 0.0)

        # chunk-persistent double buffers.
        # slab layouts: [128, block, S*Bs] with the (step, batch) index
        # innermost so matmul/transpose operands are single-free-dim slices.
        def mk_slabs(sfx):
            xT = slab_p.tile([128, NKT, S * Bs], wh_dt, tag="xT" + sfx,
                             name="xT" + sfx)  # x^T for the chunk
            cs = slab_p.tile([128, 4, S * Bs], F32, tag="cs" + sfx,
                             name="cs" + sfx)   # c0kt0,c0kt1,c1kt0,c1kt1
            hs = slab_p.tile([128, 4, S * Bs], wh_dt, tag="hs" + sfx,
                             name="hs" + sfx)   # h0kt0,h0kt1,h1kt0,h1kt1
            return xT, cs, hs
        bufA = mk_slabs("A")
        bufB = mk_slabs("B")

        def bulk(koff, xT):
            """Gather + transpose the chunk's embeddings into the xT slab.
            The x-projection itself happens inside each step's matmul
            group (x terms lead each group and pre-fill PSUM early)."""
            for tb in range(S * Bs // 128):
                idx = idx_p.tile([128, 1], I32, tag="idx", name="idx")
                nc.sync.dma_start(idx[:, 0:1],
                                  tok[bass.ds(koff * Bs + tb * 128, 128)])
                xg = xg_p.tile([128, H], emb_dt, tag="xg", name="xg")
                nc.gpsimd.indirect_dma_start(
                    out=xg[:], out_offset=None, in_=emb[:],
                    in_offset=bass.IndirectOffsetOnAxis(ap=idx[:, 0:1], axis=0))
                for kt in range(NKT):
                    pxt = pxt_p.tile([128, 128], emb_dt, tag="pxt",
                                     name="pxtb")
                    nc.tensor.transpose(pxt[:], xg[:, kt * 128:(kt + 1) * 128],
                                        ident_e[:])
                    nc.vector.tensor_copy(
                        xT[:, kt, tb * 128:(tb + 1) * 128], pxt[:])

        def xphase(pz_tag, wxname, xT, s):
            """Open step s's PSUM group with the x-projection matmuls.
            Emitted ahead of the recurrent phase so the (FIFO) PE runs them
            while the h matmuls of earlier steps are still blocked."""
            sl = slice(s * Bs, (s + 1) * Bs)
            pool = pz0_p if pz_tag == "pz0" else pz1_p
            pz = pool.tile([128, MB, Bs], F32, tag=pz_tag, name=pz_tag)
            # ONE start for the whole tile: start=True marks the entire 2KB
            # psum bank pending-zero, so each slice's first write overwrites
            # (initializes) and every later write accumulates.
            for mb in range(MB):
                for kt in range(NKT):
                    nc.tensor.matmul(
                        pz[:, mb, :],
                        lhsT=w_sb[wxname][:, (mb * NKT + kt) * 128:
                                          (mb * NKT + kt + 1) * 128],
                        rhs=xT[:, kt, sl],
                        start=(mb == 0 and kt == 0), stop=False,
                        skip_group_check=(mb or kt) != 0)
            return pz

        def cell(s, pz, zs_tag, terms, cprev, cout, hout_b, ceng=None):
            """Recurrent phase of one LSTM cell (transposed layout).
            terms: list of (w_name, rhs_fn) with rhs_fn(kt) -> [128, Bs] AP;
            accumulates onto the x-projection already in pz. ceng: engine
            for the c-update trio (L1 uses the idle GpSimd so its ops never
            queue ahead of L0's in the DVE FIFO)."""
            ceng = ceng or nc.vector
            nmm = len(terms) * NKT
            for mb in range(MB):
                mi = 0
                for wname, rhs_fn in terms:
                    for kt in range(NKT):
                        mi += 1
                        last = (mb == MB - 1 and mi == nmm)
                        nc.tensor.matmul(
                            pz[:, mb, :],
                            lhsT=w_sb[wname][:, (mb * NKT + kt) * 128:
                                             (mb * NKT + kt + 1) * 128],
                            rhs=rhs_fn(kt),
                            start=False, stop=last,
                            skip_group_check=not last)
            zs = zs_p.tile([128, MB, Bs], F32, tag=zs_tag, name=zs_tag)
            nc.scalar.activation(zs[:, :, :], pz[:, :, :], SIG)
            si, sf, so = zs[:, 0:2, :], zs[:, 2:4, :], zs[:, 4:6, :]
            tj = sm_p.tile([128, NKT, Bs], F32, tag="tj", name="tj")
            ceng.tensor_scalar(out=tj[:, :, :], in0=zs[:, 6:8, :],
                               scalar1=2.0, scalar2=-1.0,
                               op0=MULT, op1=ADD)
            v = sm_p.tile([128, NKT, Bs], F32, tag="v", name="v")
            ceng.tensor_tensor(v[:, :, :], cprev, sf, op=MULT)
            u = sm_p.tile([128, NKT, Bs], F32, tag="u", name="u")
            ceng.tensor_tensor(u[:, :, :], si, tj[:, :, :], op=MULT)
            ceng.tensor_tensor(cout, u[:, :, :], v[:, :, :], op=ADD)
            tc_ = sm_p.tile([128, NKT, Bs], F32, tag="tc", name="tc")
            nc.scalar.activation(tc_[:, :, :], cout, TANH)
            ceng.tensor_tensor(hout_b, tc_[:, :, :], so, op=MULT)

        LAG = 2

        def cell1(s, pz, cs, hs):
            """Layer-1 recurrent phase for step s (emitted LAG steps late so
            its ops are always dep-ready when the FIFO engines reach them)."""
            sl = slice(s * Bs, (s + 1) * Bs)
            if s == 0:
                c1p = carry_c1[:, :, :]
                h1p = lambda kt: carry_h1[:, kt, :]
            else:
                pv = slice((s - 1) * Bs, s * Bs)
                c1p = cs[:, 2:4, pv]
                h1p = lambda kt, pv=pv: hs[:, 2 + kt, pv]
            h0c = lambda kt, sl=sl: hs[:, kt, sl]
            cell(s, pz, "zs1",
                 [("w1m", h0c), ("w1h", h1p)],
                 c1p, cs[:, 2:4, sl], hs[:, 2:4, sl], ceng=nc.gpsimd)

        P0 = 3  # x-phase lookahead for layer 0

        def steps_chunk(koff, xT, cs, hs):
            q0, q1 = [], []
            for s in range(P0):
                q0.append(xphase("pz0", "w0x", xT, s))
            for s in range(S):
                sl = slice(s * Bs, (s + 1) * Bs)
                if s == 0:
                    c0p = carry_c0[:, :, :]
                    h0p = lambda kt: carry_h0[:, kt, :]
                else:
                    pv = slice((s - 1) * Bs, s * Bs)
                    c0p = cs[:, 0:2, pv]
                    h0p = lambda kt, pv=pv: hs[:, kt, pv]
                if s + P0 < S:
                    q0.append(xphase("pz0", "w0x", xT, s + P0))
                cell(s, q0.pop(0), "zs0",
                     [("w0h", h0p)],
                     c0p, cs[:, 0:2, sl], hs[:, 0:2, sl])
                q1.append(xphase("pz1", "w1x", xT, s))
                if s >= LAG:
                    cell1(s - LAG, q1.pop(0), cs, hs)
            for s in range(S - LAG, S):
                cell1(s, q1.pop(0), cs, hs)
            # carry into persistent tiles (split per layer)
            lastc = slice((S - 1) * Bs, S * Bs)
            nc.vector.tensor_copy(carry_c0[:], cs[:, 0:2, lastc])
            nc.vector.tensor_copy(carry_h0[:], hs[:, 0:2, lastc])
            nc.vector.tensor_copy(carry_c1[:], cs[:, 2:4, lastc])
            nc.vector.tensor_copy(carry_h1[:], hs[:, 2:4, lastc])
            # extraction: transpose c0/h0/c1 to row-major [4*S, OUT] rows
            for tb in range(S * Bs // 128):
                tsl = slice(tb * 128, (tb + 1) * 128)
                rm = rm_p.tile([128, OUT], F32, tag="rm", name="rm")
                srcs = []
                for kt in range(NKT):
                    srcs.append((cs[:, kt, tsl], kt * 128, F32))
                    srcs.append((hs[:, kt, tsl], H + kt * 128, wh_dt))
                    srcs.append((cs[:, 2 + kt, tsl], 2 * H + kt * 128, F32))
                for src, coff, sdt in srcs:
                    pxt = pxt_p.tile([128, 128], sdt, tag="pxt", name="pxt")
                    idn = ident[:] if sdt == F32 else ident_e[:]
                    nc.tensor.transpose(pxt[:], src, idn)
                    nc.vector.tensor_copy(rm[:, coff:coff + 128], pxt[:])
                nc.sync.dma_start(
                    st_rm[bass.ds((koff + tb * 32) * Bs, 128), :], rm[:])

        # ---- software-pipelined chunk loop ----
        bulk(0, bufA[0])
        with tc.For_i(0, (n_chunks - 2) * S, 2 * S,
                      hint_engines=tuple(mybir.ALL_ENGINES)) as k:
            bulk(k + S, bufB[0])
            steps_chunk(k, *bufA)
            bulk(k + 2 * S, bufA[0])
            steps_chunk(k + S, *bufB)
        bulk((n_chunks - 1) * S, bufB[0])
        steps_chunk((n_chunks - 2) * S, *bufA)
        steps_chunk((n_chunks - 1) * S, *bufB)

        # ---- output extraction ----
        nst_sb = const_p.tile([Bs, 1], I32)
        nc.sync.dma_start(nst_sb[:, 0:1], nst[:])
        iota_t = const_p.tile([Bs, 1], I32)
        nc.gpsimd.iota(iota_t[:], pattern=[[0, 1]], base=0,
                       channel_multiplier=1)
        ridx = const_p.tile([Bs, 1], I32)
        nc.vector.tensor_scalar(out=ridx[:], in0=nst_sb[:], scalar1=Bs,
                                scalar2=None, op0=MULT)
        nc.vector.tensor_tensor(ridx[:], ridx[:], iota_t[:], op=ADD)
        out_sb = const_p.tile([Bs, OUT], F32)
        nc.gpsimd.indirect_dma_start(
            out=out_sb[:], out_offset=None, in_=st_rm[:],
            in_offset=bass.IndirectOffsetOnAxis(ap=ridx[:, 0:1], axis=0))
        nc.sync.dma_start(out[:], out_sb[:])

    nc.compile()
    return nc


# ---------------------------------------------------------------------------
# Host-side entry point
# ---------------------------------------------------------------------------
import numpy as np

N_CORES = 8
_PROGRAM_CACHE = {}
_last_in_maps = None
_last_nc = None


def _plan(T):
    S = 128
    n_chunks = max(4, -(-T // S))
    if n_chunks % 2:
        n_chunks += 1
    return S * n_chunks, S, n_chunks


def _get_program(T_pad, S, n_chunks, weights, wkey):
    key = (T_pad, S, n_chunks, wkey)
    if key not in _PROGRAM_CACHE:
        _PROGRAM_CACHE[key] = build(T_pad, S, n_chunks, weights)
    return _PROGRAM_CACHE[key]


def _block_layout(w, kt_major):
    """[256, 1024] -> [128, 16*128] with 128x128 blocks at col offsets.
    kt_major: block index = kt*8+mb, else mb*2+kt."""
    t = w.reshape(NKT, 128, MB, 128)          # kt, p, mb, j
    if kt_major:
        t = t.transpose(1, 0, 2, 3)           # p, kt, mb, j
    else:
        t = t.transpose(1, 2, 0, 3)           # p, mb, kt, j
    return np.ascontiguousarray(t.reshape(128, NKT * MB * 128))


def kernel(inputs, nstarts, emb, W0, b0, W1, b1, W2, b2, _run_kwargs=None,
           _return_raw=False):
    inputs = np.asarray(inputs)
    nstarts = np.asarray(nstarts)
    emb = np.asarray(emb, np.float32)
    B, T = inputs.shape
    assert B == N_CORES * Bs
    T_pad, S, n_chunks = _plan(T)

    W0 = np.asarray(W0, np.float32)
    W1 = np.asarray(W1, np.float32)
    b0 = np.asarray(b0, np.float32)
    b1 = np.asarray(b1, np.float32)
    assert not b0.any() and not b1.any(), "nonzero biases unsupported"
    # gate perm (i,j,f,o) -> (i,f,o,j); j cols scaled by 2 (tanh via sigmoid)
    perm = np.r_[0:H, 2 * H:3 * H, 3 * H:4 * H, H:2 * H]
    jscale = np.ones(G, np.float32)
    jscale[3 * H:] = 2.0

    def prep(wrows, kt_major):
        return _block_layout(np.ascontiguousarray(wrows[:, perm] * jscale),
                             kt_major)

    import ml_dtypes
    w_maps = {
        "w0x": prep(W0[0:H], False).astype(ml_dtypes.bfloat16),
        "w0h": prep(W0[H:2 * H], False).astype(ml_dtypes.bfloat16),
        "w1x": prep(W1[0:H], False).astype(ml_dtypes.bfloat16),
        "w1m": prep(W1[H:2 * H], False).astype(ml_dtypes.bfloat16),
        "w1h": prep(W1[2 * H:3 * H], False).astype(ml_dtypes.bfloat16),
    }
    import hashlib
    wkey = hashlib.sha1(W0.tobytes() + W1.tobytes()).hexdigest()[:16]

    tt_by_batch = np.zeros(B, np.int64)
    tt_by_batch[nstarts[:, 1].astype(np.int64)] = nstarts[:, 0].astype(np.int64)

    in_maps = []
    for c in range(N_CORES):
        shard = inputs[c * Bs:(c + 1) * Bs].astype(np.int64)   # [Bs, T]
        tokp = np.zeros((T_pad, Bs), np.int64)
        tokp[:T, :] = shard.T
        uniq, inv = np.unique(tokp, return_inverse=True)
        assert len(uniq) <= VC
        emb_c = np.zeros((VC, H), ml_dtypes.bfloat16)
        emb_c[:len(uniq)] = emb[uniq].astype(ml_dtypes.bfloat16)
        in_maps.append(dict(
            tok=np.ascontiguousarray(inv.reshape(T_pad * Bs).astype(np.int32)),
            nst=np.ascontiguousarray(
                tt_by_batch[c * Bs:(c + 1) * Bs].astype(np.int32)),
            emb=emb_c,
        ))

    global _last_in_maps, _last_nc
    _last_in_maps = in_maps
    nc = _get_program(T_pad, S, n_chunks, w_maps, wkey)
    _last_nc = nc
    from concourse.bass_utils import run_bass_kernel_spmd
    kw = dict(_run_kwargs or {})
    res = run_bass_kernel_spmd(nc, in_maps, list(range(N_CORES)), **kw)
    per_batch = np.concatenate([res.results[c]["out"] for c in range(N_CORES)],
                               axis=0)                          # [B, OUT]
    full = per_batch[nstarts[:, 1].astype(np.int64)].astype(np.float32)
    if _return_raw:
        return full, res
    return full
